# revision 2
# baseline (speedup 1.0000x reference)
"""DCNv4-1D fused Trainium2 kernel v2. Data-parallel over batch N across 8 cores.

Per core (one sample [C=256, L=8192]):
  1. x streamed HBM->SBUF (loads split across the sync/gpsimd DMA rings),
     cast f32->f16 on DVE; LN stat sums via f16 ones-matmuls on PE into
     PSUM rows {0,32}.
  2. Per L/2 half: stat rows -> SBUF, selection-matmuls ([48,128].T @ sel2)
     transpose them into a [128, (c,j,sq)] PSUM column form where the
     reciprocal/sqrt math runs 128-wide; a PE transpose turns mu/rs back
     into l-ordered rows, one DMA packs them into murow_buf (DRAM), and
     partition-broadcast DMAs (to_broadcast) fan them out to mu_bc/rs_bc.
  3. LN apply: x_h -= mu_bc, u = x_h * rs_bc (both DVE),
     xa = gelu(ln_w*u + ln_b) on ACT with per-partition scale/bias.
  4. om = om_wT.T @ xa on PE (f16), +bias via ACT copy; om rows are
     host-permuted to (g, qm, k) order so the om->qm_r repack is one
     big DMA per lsub (issued on the gpsimd SWDGE ring).
  5. xa (two half tiles with edge pads) repacked to the (lsub, g) row
     layout with big window DMAs alternating sync/scalar rings; windows
     crossing the half boundary are stitched from both halves.
  6. Deformable gather = 6-diagonal banded weighted sum (exact):
     coeff[g,d,l] = sum_k mask*relu(1-|q-d|), q = off + k - 1 (grid
     folded into the om bias). Abs on ACT, rest on DVE; products f16 2x
     on DVE (one diagonal on GpSimd) with stride-0 group-broadcast;
     d-accumulation via PE identity matmuls into 4 parallel PSUM banks
     from a 7-deep rotating bank pool; f16 output staged on ACT.
"""

import json

import numpy as np

N, C, L = 8, 256, 8192
G, K, GC = 8, 3, 32
LN_EPS = 1e-6
NCT = 2
LQ = 512
NS = L // LQ             # 16 lsubs
HALO = 4
WIN = LQ + 2 * HALO      # 520
D_LO = -3
ND = 6
CPG = 4                  # channels per aggregation mult
SC = 1024                # x load superchunk
AC = 1024                # LN apply chunk
LH = L // 2              # half length

_cache = {}


# --- BIR post-pass: this walrus build rejects >1 sync wait per instruction;
# split extras onto same-engine NoOps inserted just before the owner. ---
def _split_multi_waits(bir_json: bytes, max_waits: int = 1) -> bytes:
    j = json.loads(bir_json)
    n = [0]

    def fresh():
        n[0] += 1
        return f"I-wsplit-{n[0]}"

    for fn in j.get("functions", []):
        for bb in fn.get("basicblocks", []) or fn.get("blocks", []) or []:
            out = []
            for inst in bb.get("instructions", []):
                si = inst.get("sync_info")
                waits = (si or {}).get("on_wait") or []
                if len(waits) > max_waits:
                    for w in waits[:-max_waits]:
                        out.append({
                            "debug": inst.get("debug", 0),
                            "engine": inst["engine"],
                            "ins": [], "outs": [],
                            "name": fresh(),
                            "opcode": "NoOp",
                            "sync_info": {"on_update": [], "on_wait": [w]},
                        })
                    si["on_wait"] = waits[-max_waits:]
                out.append(inst)
            bb["instructions"] = out
    return json.dumps(j).encode()


def _install_patch():
    import concourse.bass2jax as bass2jax
    import concourse.bass_utils as bass_utils

    if getattr(bass2jax.compile_bir_kernel, "_wsplit", False):
        return
    orig = bass_utils.compile_bir_kernel

    def patched(bir_json, tmpdir, neff_name="file.neff"):
        return orig(_split_multi_waits(bir_json), tmpdir, neff_name=neff_name)

    patched._wsplit = True
    bass_utils.compile_bir_kernel = patched
    bass2jax.compile_bir_kernel = patched


def _build_module():
    import contextlib

    import concourse.bass as bass
    import concourse.tile as tile
    from concourse import mybir

    f32 = mybir.dt.float32
    f16 = mybir.dt.float16
    AF = mybir.ActivationFunctionType
    OP = mybir.AluOpType

    nc = bass.Bass()

    x_d = nc.dram_tensor("x", [C, L], f32, kind="ExternalInput")
    lnw_d = nc.dram_tensor("lnw_col", [C, 1], f32, kind="ExternalInput")
    lnb_d = nc.dram_tensor("lnb_col", [C, 1], f32, kind="ExternalInput")
    onesc_d = nc.dram_tensor("onesc16", [128, 1], f16, kind="ExternalInput")
    sel2_d = nc.dram_tensor("sel2", [48, 2], f16, kind="ExternalInput")
    omwT_d = nc.dram_tensor("om_wT", [C, 2 * G * K], f16, kind="ExternalInput")
    bias48_d = nc.dram_tensor("bias48", [2 * G * K, 1], f32,
                              kind="ExternalInput")
    id128_d = nc.dram_tensor("id128h", [128, 128], f16, kind="ExternalInput")
    murow_d = nc.dram_tensor("murow_buf", [2, L], f16, kind="Internal")
    # out rows (s,g), cols (cp, j, f): host unshuffles to [C, L]
    out_d = nc.dram_tensor("out", [128, GC * LQ], f16, kind="ExternalOutput")

    with tile.TileContext(nc) as tc, contextlib.ExitStack() as ctx:
        const = ctx.enter_context(tc.tile_pool(name="const", bufs=1))
        persist = ctx.enter_context(tc.tile_pool(name="persist", bufs=1))
        sqp = ctx.enter_context(tc.tile_pool(name="sqp", bufs=2))
        sqsbp = ctx.enter_context(tc.tile_pool(name="sqsbp", bufs=2))
        statp = ctx.enter_context(tc.tile_pool(name="statp", bufs=1))
        bcp = ctx.enter_context(tc.tile_pool(name="bcp", bufs=2))
        tup = ctx.enter_context(tc.tile_pool(name="tup", bufs=2))
        coefp = ctx.enter_context(tc.tile_pool(name="coefp", bufs=2))
        cdp = ctx.enter_context(tc.tile_pool(name="cdp", bufs=1))
        tmpp = ctx.enter_context(tc.tile_pool(name="tmpp", bufs=3))
        outp = ctx.enter_context(tc.tile_pool(name="outp", bufs=2))
        # PSUM: bank pool (stats sums / om / agg accs) 6 + st 1 + mrT 1 = 8
        psum_b = ctx.enter_context(
            tc.tile_pool(name="psum_b", bufs=6, space="PSUM"))
        psum_s = ctx.enter_context(
            tc.tile_pool(name="psum_s", bufs=1, space="PSUM"))

        # ---------------- constants ----------------
        lnw_c, lnb_c, omwT = [], [], []
        for ct in range(NCT):
            t = const.tile([128, 1], f32, tag=f"lnw{ct}", name=f"lnw{ct}")
            nc.sync.dma_start(out=t, in_=lnw_d[ct * 128:(ct + 1) * 128, :])
            lnw_c.append(t)
            t = const.tile([128, 1], f32, tag=f"lnb{ct}", name=f"lnb{ct}")
            nc.sync.dma_start(out=t, in_=lnb_d[ct * 128:(ct + 1) * 128, :])
            lnb_c.append(t)
            t = const.tile([128, 2 * G * K], f16, tag=f"omwT{ct}",
                           name=f"omwT{ct}")
            nc.scalar.dma_start(out=t, in_=omwT_d[ct * 128:(ct + 1) * 128, :])
            omwT.append(t)
        onesc = const.tile([128, 1], f16, tag="onesc", name="onesc")
        nc.sync.dma_start(out=onesc, in_=onesc_d[:])
        sel2 = const.tile([48, 2], f16, tag="sel2", name="sel2")
        nc.sync.dma_start(out=sel2, in_=sel2_d[:])
        bias48 = const.tile([2 * G * K, 1], f32, tag="bias48", name="bias48")
        nc.scalar.dma_start(out=bias48, in_=bias48_d[:])
        id128 = const.tile([128, 128], f16, tag="id128", name="id128")
        nc.scalar.dma_start(out=id128, in_=id128_d[:])
        eps_c = const.tile([128, 1], f32, tag="eps_c", name="eps_c")
        nc.vector.memset(eps_c, LN_EPS)
        negd_c = []
        for i in range(ND):
            t = const.tile([128, 1], f32, tag=f"negd{i}", name=f"negd{i}")
            nc.vector.memset(t, float(-(D_LO + i)))
            negd_c.append(t)
        # zero both sq_sb rotation buffers once: the per-chunk copies only
        # write rows {0,32}; the other rows feed the sel2 matmul and must
        # not hold garbage (first-run PSUM/SBUF garbage can be NaN, and
        # NaN * 0 poisons the stats)
        for i in range(2):
            z = sqsbp.tile([48, LQ], f16, tag="sq_sb", name=f"sq_sb_z{i}")
            nc.vector.memset(z, 0.0)

        # ---------------- persistent tensors ----------------
        x_h = [persist.tile([128, L], f16, tag=f"x_h{ct}", name=f"x_h{ct}")
               for ct in range(NCT)]
        # xa half tiles: h0 covers l in [-4, 4096), h1 covers [4096, 8196)
        xa = [[persist.tile([128, LH + HALO], f16, tag=f"xa{ct}_{h}",
                            name=f"xa{ct}_{h}") for h in range(2)]
              for ct in range(NCT)]
        xa_r = persist.tile([128, GC * WIN], f16, tag="xa_r", name="xa_r")
        qm_r = persist.tile([128, 2 * K * LQ], f16, tag="qm_r", name="qm_r")

        for ct in range(NCT):
            nc.vector.memset(xa[ct][0][:, 0:HALO], 0.0)
            nc.vector.memset(xa[ct][1][:, LH:LH + HALO], 0.0)

        def xa_ap(ct, gl0, width):
            """AP for xa unpadded cols [gl0, gl0+width); no half crossing."""
            if gl0 < LH:
                return xa[ct][0][:, gl0 + HALO:gl0 + HALO + width]
            return xa[ct][1][:, gl0 - LH:gl0 - LH + width]

        # prefetch-load all of x as f16 (SWDGE cast DMA, 8KB src runs)
        for lo in range(0, L, 2048):
            for ct in range(NCT):
                nc.gpsimd.dma_start(
                    out=x_h[ct][:, lo:lo + 2048],
                    in_=x_d[ct * 128:(ct + 1) * 128, lo:lo + 2048])

        # ------------- per-half phases -------------
        def emit_stats_chunk(st_ps, h, ci):
            """stat sums + stat column transposes for chunk ci of half h"""
            cf = h * LH + ci * LQ
            sq_ps = psum_b.tile([128, LQ], f32, tag="bank", name="sq_ps")
            for ct in range(NCT):
                nc.tensor.matmul(sq_ps[0:1, :], onesc,
                                 x_h[ct][:, cf:cf + LQ],
                                 start=(ct == 0), stop=(ct == NCT - 1))
            for ct in range(NCT):
                xsq = sqp.tile([128, LQ], f16, tag=f"xsq{ct}",
                               name=f"xsq{ct}")
                nc.vector.tensor_tensor(
                    out=xsq, in0=x_h[ct][:, cf:cf + LQ],
                    in1=x_h[ct][:, cf:cf + LQ], op=OP.mult)
                nc.tensor.matmul(sq_ps[32:33, :], onesc, xsq,
                                 start=(ct == 0), stop=(ct == NCT - 1))
            sq_sb = sqsbp.tile([48, LQ], f16, tag="sq_sb", name="sq_sb")
            nc.scalar.copy(out=sq_sb[0:1, :], in_=sq_ps[0:1, :])
            nc.scalar.copy(out=sq_sb[32:33, :], in_=sq_ps[32:33, :])
            for j in range(4):
                nc.tensor.matmul(
                    st_ps[:, (ci * 4 + j) * 2:(ci * 4 + j) * 2 + 2],
                    sq_sb[:, j * 128:(j + 1) * 128], sel2,
                    start=True, stop=True)

        def phase_statmath(h, st_ps):
            """stat math in column form; writes murow_d cols of half h.
            st cols (c,j,sq): l = h*LH + c*512 + j*128 + p"""
            part = st_ps[:].ap[0]
            s_v = bass.AP(tensor=st_ps.tensor, offset=st_ps.offset,
                          ap=[part, [8, 8], [2, 4]])
            q_v = bass.AP(tensor=st_ps.tensor, offset=st_ps.offset + 1,
                          ap=[part, [8, 8], [2, 4]])
            mu32 = statp.tile([128, 32], f32, tag="mu32", name="mu32")
            nc.vector.tensor_scalar_mul(out=mu32[:].rearrange(
                "p (c j) -> p c j", j=4), in0=s_v, scalar1=1.0 / C)
            mu2 = statp.tile([128, 32], f32, tag="mu2", name="mu2")
            nc.vector.tensor_tensor(out=mu2, in0=mu32, in1=mu32, op=OP.mult)
            var32 = statp.tile([128, 32], f32, tag="var32", name="var32")
            nc.vector.scalar_tensor_tensor(
                out=var32[:].rearrange("p (c j) -> p c j", j=4), in0=q_v,
                scalar=1.0 / C, in1=mu2[:].rearrange("p (c j) -> p c j", j=4),
                op0=OP.mult, op1=OP.subtract)
            sd32 = statp.tile([128, 32], f32, tag="sd32", name="sd32")
            nc.scalar.activation(out=sd32, in_=var32, func=AF.Sqrt,
                                 bias=eps_c, scale=1.0)
            rs32 = statp.tile([128, 32], f32, tag="rs32", name="rs32")
            nc.vector.reciprocal(out=rs32, in_=sd32)
            mr = statp.tile([128, 64], f16, tag="mr", name="mr")
            nc.vector.tensor_scalar_mul(out=mr[:, 0:32], in0=mu32,
                                        scalar1=1.0)
            nc.vector.tensor_scalar_mul(out=mr[:, 32:64], in0=rs32,
                                        scalar1=1.0)
            mrT_ps = psum_s.tile([64, 128], f16, tag="mrT", name="mrT")
            nc.tensor.transpose(mrT_ps, mr, id128)
            mr_rows = statp.tile([64, 128], f16, tag="mr_rows",
                                 name="mr_rows")
            nc.scalar.copy(out=mr_rows, in_=mrT_ps)
            nc.sync.dma_start(out=murow_d[0:1, h * LH:(h + 1) * LH],
                              in_=mr_rows[0:32, :])
            nc.sync.dma_start(out=murow_d[1:2, h * LH:(h + 1) * LH],
                              in_=mr_rows[32:64, :])

        # xa -> xa_r window repack; row = s*8+g, free (cc, w);
        # window = unpadded l in [s*512-4, s*512+516)
        def emit_xar(s):
            eng = nc.sync if s % 2 == 0 else nc.scalar
            a0 = s * LQ - HALO                      # global start (unpadded)
            if a0 + WIN <= LH:
                pieces = [(0, WIN, 0)]              # (w-offset, width, half)
            elif a0 >= LH:
                pieces = [(0, WIN, 1)]
            else:
                w0 = LH - a0
                pieces = [(0, w0, 0), (w0, WIN - w0, 1)]
            for ct in range(NCT):
                r0 = s * 8 + 4 * ct
                for (wo, wl, hh) in pieces:
                    b = a0 + wo
                    src = (xa[ct][0][:, b + HALO:b + HALO + wl] if hh == 0
                           else xa[ct][1][:, b - LH:b - LH + wl])
                    eng.dma_start(
                        out=xa_r[r0:r0 + 4, :].rearrange(
                            "g (cc w) -> g cc w",
                            cc=GC)[:, :, wo:wo + wl],
                        in_=src)

        BCH = 2048

        def emit_apply_block(h, b):
            if True:
                blo = h * LH + b * BCH
                mu_bc = bcp.tile([128, BCH], f16, tag="mu_bc", name="mu_bc")
                nc.gpsimd.dma_start(
                    out=mu_bc,
                    in_=murow_d[0:1, blo:blo + BCH].to_broadcast((128, BCH)))
                rs_bc = bcp.tile([128, BCH], f16, tag="rs_bc", name="rs_bc")
                nc.gpsimd.dma_start(
                    out=rs_bc,
                    in_=murow_d[1:2, blo:blo + BCH].to_broadcast((128, BCH)))
                for a in range(BCH // AC):
                    lo = blo + a * AC
                    la = a * AC
                    for ct in range(NCT):
                        nc.vector.tensor_tensor(
                            out=x_h[ct][:, lo:lo + AC],
                            in0=x_h[ct][:, lo:lo + AC],
                            in1=mu_bc[:, la:la + AC],
                            op=OP.subtract)
                        u_t = tup.tile([128, AC], f16, tag=f"u{ct}",
                                       name=f"u{ct}")
                        nc.vector.tensor_tensor(out=u_t,
                                                in0=x_h[ct][:, lo:lo + AC],
                                                in1=rs_bc[:, la:la + AC],
                                                op=OP.mult)
                        nc.scalar.activation(
                            out=xa_ap(ct, lo, AC), in_=u_t,
                            func=AF.Gelu, bias=lnb_c[ct], scale=lnw_c[ct])
                    for c in range(AC // LQ):
                        cf = lo + c * LQ
                        s_idx = cf // LQ
                        om_ps = psum_b.tile([128, LQ], f32, tag="bank",
                                            name="om_ps")
                        for ct in range(NCT):
                            nc.tensor.matmul(
                                om_ps[0:48, :], omwT[ct], xa_ap(ct, cf, LQ),
                                start=(ct == 0), stop=(ct == NCT - 1))
                        om_sb = sqsbp.tile([2 * G * K, LQ], f16, tag="om_sb",
                                           name="om_sb")
                        nc.scalar.activation(out=om_sb, in_=om_ps[0:48, :],
                                             func=AF.Identity,
                                             bias=bias48, scale=1.0)
                        # om -> qm_r repack for this lsub (one DMA)
                        nc.scalar.dma_start(
                            out=qm_r[s_idx * 8:(s_idx + 1) * 8, :].rearrange(
                                "g (qm k w) -> g qm k w", qm=2, k=K),
                            in_=om_sb)

        # ------------- banded coefficients (per half of the s rows) ------
        KL = K * LQ
        c_d = [cdp.tile([128, LQ], f16, tag=f"c{i}", name=f"c{i}")
               for i in range(ND)]

        def coeff_d(h, i):
            p0, p1 = (0, 128) if h is None else (h * 64, (h + 1) * 64)
            if True:
                a_t = coefp.tile([128, KL], f16, tag="a_t", name="a_t")
                nc.scalar.activation(out=a_t[p0:p1, :],
                                     in_=qm_r[p0:p1, 0:KL], func=AF.Abs,
                                     bias=negd_c[i][p0:p1, :], scale=1.0)
                nc.vector.tensor_scalar(out=a_t[p0:p1, :],
                                        in0=a_t[p0:p1, :], scalar1=-1.0,
                                        scalar2=1.0, op0=OP.mult, op1=OP.add)
                w_t = coefp.tile([128, KL], f16, tag="w_t", name="w_t")
                nc.vector.scalar_tensor_tensor(
                    out=w_t[p0:p1, :], in0=a_t[p0:p1, :], scalar=0.0,
                    in1=qm_r[p0:p1, KL:2 * KL], op0=OP.max, op1=OP.mult)
                nc.vector.tensor_add(out=c_d[i][p0:p1, :],
                                     in0=w_t[p0:p1, 0:LQ],
                                     in1=w_t[p0:p1, LQ:2 * LQ])
                nc.vector.tensor_add(out=c_d[i][p0:p1, :],
                                     in0=c_d[i][p0:p1, :],
                                     in1=w_t[p0:p1, 2 * LQ:3 * LQ])

        # ------------- phase 1: software-pipelined halves -------------
        st0 = psum_s.tile([128, 64], f32, tag="st", name="st0")
        for ci in range(8):
            emit_stats_chunk(st0, 0, ci)
        phase_statmath(0, st0)
        st1 = psum_s.tile([128, 64], f32, tag="st", name="st1")
        # apply h0 interleaved with stats h1
        rep_done = 0
        for b in range(2):
            emit_apply_block(0, b)
            for k in range(4):
                emit_stats_chunk(st1, 1, 4 * b + k)
            smax = ((b + 1) * BCH - WIN + HALO) // LQ + 1
            for s in range(rep_done, smax):
                emit_xar(s)
            rep_done = smax
        phase_statmath(1, st1)
        # apply h1
        for b in range(2):
            emit_apply_block(1, b)
            smax = ((LH + (b + 1) * BCH - WIN + HALO) // LQ + 1
                    if b == 0 else NS)
            for s in range(rep_done, smax):
                emit_xar(s)
            rep_done = smax
        for i in range(ND):
            coeff_d(None, i)

        # ------------- phase 4: banded aggregation -------------
        xa_r_v = xa_r[:].rearrange("p (cc w) -> p cc w", cc=GC)
        for cp in range(GC // CPG):
            accs = [psum_b.tile([128, LQ], f32, tag="bank", name=f"acc{j}")
                    for j in range(CPG)]
            for i in range(ND):
                d = D_LO + i
                tmp = tmpp.tile([128, CPG * LQ], f16, tag="tmp", name="tmp")
                cb = bass.AP(tensor=c_d[i].tensor, offset=c_d[i].offset,
                             ap=[c_d[i][:].ap[0], [0, CPG], [1, LQ]])
                nc.vector.tensor_tensor(
                    out=tmp[:].rearrange("p (c f) -> p c f", c=CPG),
                    in0=xa_r_v[:, cp * CPG:(cp + 1) * CPG,
                               HALO + d:HALO + d + LQ],
                    in1=cb, op=OP.mult)
                for j in range(CPG):
                    nc.tensor.matmul(accs[j], id128,
                                     tmp[:, j * LQ:(j + 1) * LQ],
                                     start=(i == 0), stop=(i == ND - 1))
            outc = outp.tile([128, CPG * LQ], f16, tag="outc", name="outc")
            for j in range(CPG):
                nc.scalar.copy(out=outc[:, j * LQ:(j + 1) * LQ],
                               in_=accs[j])
            nc.gpsimd.dma_start(
                out=out_d[:, cp * CPG * LQ:(cp + 1) * CPG * LQ], in_=outc)

    return nc


def _host_params(ln_w, ln_b, om_w, om_b):
    # om rows permuted to (g, qm, k); bias includes the conv grid (k-1)
    # folded into the offset rows.
    perm = np.zeros(2 * G * K, dtype=np.int64)
    bias = np.zeros(2 * G * K, dtype=np.float32)
    for g in range(G):
        for qm in range(2):
            for k in range(K):
                row = g * 6 + qm * 3 + k
                orig = qm * G * K + g * K + k
                perm[row] = orig
                bias[row] = om_b[orig] + (k - 1.0 if qm == 0 else 0.0)
    omP = om_w[perm]  # [48, C]
    sel2 = np.zeros((48, 2), np.float16)
    sel2[0, 0] = 1.0
    sel2[32, 1] = 1.0
    return {
        "lnw_col": np.asarray(ln_w, np.float32).reshape(C, 1),
        "lnb_col": np.asarray(ln_b, np.float32).reshape(C, 1),
        "onesc16": np.ones((128, 1), np.float16),
        "sel2": sel2,
        "om_wT": np.ascontiguousarray(omP.T).astype(np.float16),
        "bias48": bias.reshape(2 * G * K, 1),
        "id128h": np.eye(128, dtype=np.float16),
    }


def _unshuffle(buf):
    # buf [128, 16384] rows (s,g), cols (cp, j, f) -> [C, L]
    return np.ascontiguousarray(
        buf.reshape(NS, G, G, CPG, LQ).transpose(1, 2, 3, 0, 4)
        .reshape(C, L))


def kernel(x, ln_w, ln_b, om_w, om_b):
    _install_patch()
    from concourse.bass_utils import run_bass_kernel_spmd

    if "nc" not in _cache:
        _cache["nc"] = _build_module()
    nc = _cache["nc"]

    x = np.ascontiguousarray(np.asarray(x, dtype=np.float32))
    params = _host_params(np.asarray(ln_w, np.float32),
                          np.asarray(ln_b, np.float32),
                          np.asarray(om_w, np.float32),
                          np.asarray(om_b, np.float32))
    in_maps = [dict(params, x=x[n]) for n in range(N)]
    for _attempt in range(3):
        res = run_bass_kernel_spmd(nc, in_maps, core_ids=list(range(N)))
        out = np.stack([_unshuffle(res.results[n]["out"])
                        for n in range(N)], axis=0).astype(np.float32)
        if np.isfinite(out).all():
            return out
    return out


def run_traced(inputs):
    _install_patch()
    from concourse.bass_utils import run_bass_kernel_spmd
    if "nc" not in _cache:
        _cache["nc"] = _build_module()
    x = np.ascontiguousarray(np.asarray(inputs["x"], dtype=np.float32))
    params = _host_params(np.asarray(inputs["ln_w"], np.float32),
                          np.asarray(inputs["ln_b"], np.float32),
                          np.asarray(inputs["om_w"], np.float32),
                          np.asarray(inputs["om_b"], np.float32))
    in_maps = [dict(params, x=x[n]) for n in range(N)]
    return run_bass_kernel_spmd(_cache["nc"], in_maps,
                                core_ids=list(range(N)), trace=True)


# revision 3
# speedup vs baseline: 1.0137x; 1.0137x over previous
"""DCNv4-1D fused Trainium2 kernel v2. Data-parallel over batch N across 8 cores.

Per core (one sample [C=256, L=8192]):
  1. x streamed HBM->SBUF (loads split across the sync/gpsimd DMA rings),
     cast f32->f16 on DVE; LN stat sums via f16 ones-matmuls on PE into
     PSUM rows {0,32}.
  2. Per L/2 half: stat rows -> SBUF, selection-matmuls ([48,128].T @ sel2)
     transpose them into a [128, (c,j,sq)] PSUM column form where the
     reciprocal/sqrt math runs 128-wide; a PE transpose turns mu/rs back
     into l-ordered rows, one DMA packs them into murow_buf (DRAM), and
     partition-broadcast DMAs (to_broadcast) fan them out to mu_bc/rs_bc.
  3. LN apply: x_h -= mu_bc, u = x_h * rs_bc (both DVE),
     xa = gelu(ln_w*u + ln_b) on ACT with per-partition scale/bias.
  4. om = om_wT.T @ xa on PE (f16), +bias via ACT copy; om rows are
     host-permuted to (g, qm, k) order so the om->qm_r repack is one
     big DMA per lsub (issued on the gpsimd SWDGE ring).
  5. xa (two half tiles with edge pads) repacked to the (lsub, g) row
     layout with big window DMAs alternating sync/scalar rings; windows
     crossing the half boundary are stitched from both halves.
  6. Deformable gather = 6-diagonal banded weighted sum (exact):
     coeff[g,d,l] = sum_k mask*relu(1-|q-d|), q = off + k - 1 (grid
     folded into the om bias). Abs on ACT, rest on DVE; products f16 2x
     on DVE (one diagonal on GpSimd) with stride-0 group-broadcast;
     d-accumulation via PE identity matmuls into 4 parallel PSUM banks
     from a 7-deep rotating bank pool; f16 output staged on ACT.
"""

import json

import numpy as np

N, C, L = 8, 256, 8192
G, K, GC = 8, 3, 32
LN_EPS = 1e-6
NCT = 2
LQ = 512
NS = L // LQ             # 16 lsubs
HALO = 4
WIN = LQ + 2 * HALO      # 520
D_LO = -3
ND = 6
CPG = 4                  # channels per aggregation mult
SC = 1024                # x load superchunk
AC = 1024                # LN apply chunk
LH = L // 2              # half length

_cache = {}


# --- BIR post-pass: this walrus build rejects >1 sync wait per instruction;
# split extras onto same-engine NoOps inserted just before the owner. ---
def _split_multi_waits(bir_json: bytes, max_waits: int = 1) -> bytes:
    j = json.loads(bir_json)
    n = [0]

    def fresh():
        n[0] += 1
        return f"I-wsplit-{n[0]}"

    for fn in j.get("functions", []):
        for bb in fn.get("basicblocks", []) or fn.get("blocks", []) or []:
            out = []
            for inst in bb.get("instructions", []):
                si = inst.get("sync_info")
                waits = (si or {}).get("on_wait") or []
                if len(waits) > max_waits:
                    for w in waits[:-max_waits]:
                        out.append({
                            "debug": inst.get("debug", 0),
                            "engine": inst["engine"],
                            "ins": [], "outs": [],
                            "name": fresh(),
                            "opcode": "NoOp",
                            "sync_info": {"on_update": [], "on_wait": [w]},
                        })
                    si["on_wait"] = waits[-max_waits:]
                out.append(inst)
            bb["instructions"] = out
    return json.dumps(j).encode()


def _install_patch():
    import concourse.bass2jax as bass2jax
    import concourse.bass_utils as bass_utils

    if getattr(bass2jax.compile_bir_kernel, "_wsplit", False):
        return
    orig = bass_utils.compile_bir_kernel

    def patched(bir_json, tmpdir, neff_name="file.neff"):
        return orig(_split_multi_waits(bir_json), tmpdir, neff_name=neff_name)

    patched._wsplit = True
    bass_utils.compile_bir_kernel = patched
    bass2jax.compile_bir_kernel = patched


def _build_module():
    import contextlib

    import concourse.bass as bass
    import concourse.tile as tile
    from concourse import mybir

    f32 = mybir.dt.float32
    f16 = mybir.dt.float16
    AF = mybir.ActivationFunctionType
    OP = mybir.AluOpType

    nc = bass.Bass()

    x_d = nc.dram_tensor("x", [C, L], f32, kind="ExternalInput")
    lnw_d = nc.dram_tensor("lnw_col", [C, 1], f32, kind="ExternalInput")
    lnb_d = nc.dram_tensor("lnb_col", [C, 1], f32, kind="ExternalInput")
    onesc_d = nc.dram_tensor("onesc16", [128, 1], f16, kind="ExternalInput")
    sel2_d = nc.dram_tensor("sel2", [48, 2], f16, kind="ExternalInput")
    omwT_d = nc.dram_tensor("om_wT", [C, 2 * G * K], f16, kind="ExternalInput")
    bias48_d = nc.dram_tensor("bias48", [2 * G * K, 1], f32,
                              kind="ExternalInput")
    id128_d = nc.dram_tensor("id128h", [128, 128], f16, kind="ExternalInput")
    murow_d = nc.dram_tensor("murow_buf", [2, L], f16, kind="Internal")
    # out rows (s,g), cols (cp, j, f): host unshuffles to [C, L]
    out_d = nc.dram_tensor("out", [128, GC * LQ], f16, kind="ExternalOutput")

    with tile.TileContext(nc) as tc, contextlib.ExitStack() as ctx:
        const = ctx.enter_context(tc.tile_pool(name="const", bufs=1))
        persist = ctx.enter_context(tc.tile_pool(name="persist", bufs=1))
        sqp = ctx.enter_context(tc.tile_pool(name="sqp", bufs=2))
        sqsbp = ctx.enter_context(tc.tile_pool(name="sqsbp", bufs=2))
        statp = ctx.enter_context(tc.tile_pool(name="statp", bufs=1))
        bcp = ctx.enter_context(tc.tile_pool(name="bcp", bufs=2))
        tup = ctx.enter_context(tc.tile_pool(name="tup", bufs=2))
        coefp = ctx.enter_context(tc.tile_pool(name="coefp", bufs=2))
        cdp = ctx.enter_context(tc.tile_pool(name="cdp", bufs=1))
        tmpp = ctx.enter_context(tc.tile_pool(name="tmpp", bufs=3))
        outp = ctx.enter_context(tc.tile_pool(name="outp", bufs=2))
        # PSUM: bank pool (stats sums / om / agg accs) 6 + st 1 + mrT 1 = 8
        psum_b = ctx.enter_context(
            tc.tile_pool(name="psum_b", bufs=6, space="PSUM"))
        psum_s = ctx.enter_context(
            tc.tile_pool(name="psum_s", bufs=1, space="PSUM"))

        # ---------------- constants ----------------
        lnw_c, lnb_c, omwT = [], [], []
        for ct in range(NCT):
            t = const.tile([128, 1], f32, tag=f"lnw{ct}", name=f"lnw{ct}")
            nc.sync.dma_start(out=t, in_=lnw_d[ct * 128:(ct + 1) * 128, :])
            lnw_c.append(t)
            t = const.tile([128, 1], f32, tag=f"lnb{ct}", name=f"lnb{ct}")
            nc.sync.dma_start(out=t, in_=lnb_d[ct * 128:(ct + 1) * 128, :])
            lnb_c.append(t)
            t = const.tile([128, 2 * G * K], f16, tag=f"omwT{ct}",
                           name=f"omwT{ct}")
            nc.scalar.dma_start(out=t, in_=omwT_d[ct * 128:(ct + 1) * 128, :])
            omwT.append(t)
        onesc = const.tile([128, 1], f16, tag="onesc", name="onesc")
        nc.sync.dma_start(out=onesc, in_=onesc_d[:])
        onesr = const.tile([1, 128], f16, tag="onesr", name="onesr")
        nc.vector.memset(onesr, 1.0)
        sel2 = const.tile([48, 2], f16, tag="sel2", name="sel2")
        nc.sync.dma_start(out=sel2, in_=sel2_d[:])
        bias48 = const.tile([2 * G * K, 1], f32, tag="bias48", name="bias48")
        nc.scalar.dma_start(out=bias48, in_=bias48_d[:])
        id128 = const.tile([128, 128], f16, tag="id128", name="id128")
        nc.scalar.dma_start(out=id128, in_=id128_d[:])
        eps_c = const.tile([128, 1], f32, tag="eps_c", name="eps_c")
        nc.vector.memset(eps_c, LN_EPS)
        negd_c = []
        for i in range(ND):
            t = const.tile([128, 1], f32, tag=f"negd{i}", name=f"negd{i}")
            nc.vector.memset(t, float(-(D_LO + i)))
            negd_c.append(t)
        # zero both sq_sb rotation buffers once: the per-chunk copies only
        # write rows {0,32}; the other rows feed the sel2 matmul and must
        # not hold garbage (first-run PSUM/SBUF garbage can be NaN, and
        # NaN * 0 poisons the stats)
        for i in range(2):
            z = sqsbp.tile([48, LQ], f16, tag="sq_sb", name=f"sq_sb_z{i}")
            nc.vector.memset(z, 0.0)

        # ---------------- persistent tensors ----------------
        x_h = [persist.tile([128, L], f16, tag=f"x_h{ct}", name=f"x_h{ct}")
               for ct in range(NCT)]
        # xa half tiles: h0 covers l in [-4, 4096), h1 covers [4096, 8196)
        xa = [[persist.tile([128, LH + HALO], f16, tag=f"xa{ct}_{h}",
                            name=f"xa{ct}_{h}") for h in range(2)]
              for ct in range(NCT)]
        xa_r = persist.tile([128, GC * WIN], f16, tag="xa_r", name="xa_r")
        murow_sb = [persist.tile([1, L], f16, tag=f"murow{r}",
                                 name=f"murow{r}") for r in range(2)]
        qm_r = persist.tile([128, 2 * K * LQ], f16, tag="qm_r", name="qm_r")

        for ct in range(NCT):
            nc.vector.memset(xa[ct][0][:, 0:HALO], 0.0)
            nc.vector.memset(xa[ct][1][:, LH:LH + HALO], 0.0)

        def xa_ap(ct, gl0, width):
            """AP for xa unpadded cols [gl0, gl0+width); no half crossing."""
            if gl0 < LH:
                return xa[ct][0][:, gl0 + HALO:gl0 + HALO + width]
            return xa[ct][1][:, gl0 - LH:gl0 - LH + width]

        # prefetch-load all of x as f16 (SWDGE cast DMA, 8KB src runs)
        for lo in range(0, L, 2048):
            for ct in range(NCT):
                nc.gpsimd.dma_start(
                    out=x_h[ct][:, lo:lo + 2048],
                    in_=x_d[ct * 128:(ct + 1) * 128, lo:lo + 2048])

        # ------------- per-half phases -------------
        def emit_stats_chunk(st_ps, h, ci):
            """stat sums + stat column transposes for chunk ci of half h"""
            cf = h * LH + ci * LQ
            sq_ps = psum_b.tile([128, LQ], f32, tag="bank", name="sq_ps")
            for ct in range(NCT):
                nc.tensor.matmul(sq_ps[0:1, :], onesc,
                                 x_h[ct][:, cf:cf + LQ],
                                 start=(ct == 0), stop=(ct == NCT - 1))
            for ct in range(NCT):
                xsq = sqp.tile([128, LQ], f16, tag=f"xsq{ct}",
                               name=f"xsq{ct}")
                nc.vector.tensor_tensor(
                    out=xsq, in0=x_h[ct][:, cf:cf + LQ],
                    in1=x_h[ct][:, cf:cf + LQ], op=OP.mult)
                nc.tensor.matmul(sq_ps[32:33, :], onesc, xsq,
                                 start=(ct == 0), stop=(ct == NCT - 1))
            sq_sb = sqsbp.tile([48, LQ], f16, tag="sq_sb", name="sq_sb")
            nc.scalar.copy(out=sq_sb[0:1, :], in_=sq_ps[0:1, :])
            nc.scalar.copy(out=sq_sb[32:33, :], in_=sq_ps[32:33, :])
            for j in range(4):
                nc.tensor.matmul(
                    st_ps[:, (ci * 4 + j) * 2:(ci * 4 + j) * 2 + 2],
                    sq_sb[:, j * 128:(j + 1) * 128], sel2,
                    start=True, stop=True)

        def phase_statmath(h, st_ps):
            """stat math in column form; writes murow_d cols of half h.
            st cols (c,j,sq): l = h*LH + c*512 + j*128 + p"""
            part = st_ps[:].ap[0]
            s_v = bass.AP(tensor=st_ps.tensor, offset=st_ps.offset,
                          ap=[part, [8, 8], [2, 4]])
            q_v = bass.AP(tensor=st_ps.tensor, offset=st_ps.offset + 1,
                          ap=[part, [8, 8], [2, 4]])
            mu32 = statp.tile([128, 32], f32, tag="mu32", name="mu32")
            nc.vector.tensor_scalar_mul(out=mu32[:].rearrange(
                "p (c j) -> p c j", j=4), in0=s_v, scalar1=1.0 / C)
            mu2 = statp.tile([128, 32], f32, tag="mu2", name="mu2")
            nc.vector.tensor_tensor(out=mu2, in0=mu32, in1=mu32, op=OP.mult)
            var32 = statp.tile([128, 32], f32, tag="var32", name="var32")
            nc.vector.scalar_tensor_tensor(
                out=var32[:].rearrange("p (c j) -> p c j", j=4), in0=q_v,
                scalar=1.0 / C, in1=mu2[:].rearrange("p (c j) -> p c j", j=4),
                op0=OP.mult, op1=OP.subtract)
            sd32 = statp.tile([128, 32], f32, tag="sd32", name="sd32")
            nc.scalar.activation(out=sd32, in_=var32, func=AF.Sqrt,
                                 bias=eps_c, scale=1.0)
            rs32 = statp.tile([128, 32], f32, tag="rs32", name="rs32")
            nc.vector.reciprocal(out=rs32, in_=sd32)
            mr = statp.tile([128, 64], f16, tag="mr", name="mr")
            nc.vector.tensor_scalar_mul(out=mr[:, 0:32], in0=mu32,
                                        scalar1=1.0)
            nc.vector.tensor_scalar_mul(out=mr[:, 32:64], in0=rs32,
                                        scalar1=1.0)
            mrT_ps = psum_s.tile([64, 128], f16, tag="mrT", name="mrT")
            nc.tensor.transpose(mrT_ps, mr, id128)
            mr_rows = statp.tile([64, 128], f16, tag="mr_rows",
                                 name="mr_rows")
            nc.scalar.copy(out=mr_rows, in_=mrT_ps)
            nc.sync.dma_start(out=murow_sb[0][:, h * LH:(h + 1) * LH],
                              in_=mr_rows[0:32, :])
            nc.sync.dma_start(out=murow_sb[1][:, h * LH:(h + 1) * LH],
                              in_=mr_rows[32:64, :])

        # xa -> xa_r window repack; row = s*8+g, free (cc, w);
        # window = unpadded l in [s*512-4, s*512+516)
        def emit_xar(s):
            eng = nc.gpsimd
            a0 = s * LQ - HALO                      # global start (unpadded)
            if a0 + WIN <= LH:
                pieces = [(0, WIN, 0)]              # (w-offset, width, half)
            elif a0 >= LH:
                pieces = [(0, WIN, 1)]
            else:
                w0 = LH - a0
                pieces = [(0, w0, 0), (w0, WIN - w0, 1)]
            for ct in range(NCT):
                r0 = s * 8 + 4 * ct
                for (wo, wl, hh) in pieces:
                    b = a0 + wo
                    src = (xa[ct][0][:, b + HALO:b + HALO + wl] if hh == 0
                           else xa[ct][1][:, b - LH:b - LH + wl])
                    eng.dma_start(
                        out=xa_r[r0:r0 + 4, :].rearrange(
                            "g (cc w) -> g cc w",
                            cc=GC)[:, :, wo:wo + wl],
                        in_=src)

        BCH = 2048

        def emit_apply_block(h, b):
            if True:
                blo = h * LH + b * BCH
                for a in range(BCH // AC):
                    lo = blo + a * AC
                    # broadcast mu/rs rows to all partitions via PE
                    mu_ps2, rs_ps2 = [], []
                    for c in range(AC // LQ):
                        cf = lo + c * LQ
                        mu_ps = psum_b.tile([128, LQ], f32, tag="bank",
                                            name="mu_ps")
                        nc.tensor.matmul(mu_ps, onesr,
                                         murow_sb[0][:, cf:cf + LQ],
                                         start=True, stop=True)
                        mu_ps2.append(mu_ps)
                        rs_ps = psum_b.tile([128, LQ], f32, tag="bank",
                                            name="rs_ps")
                        nc.tensor.matmul(rs_ps, onesr,
                                         murow_sb[1][:, cf:cf + LQ],
                                         start=True, stop=True)
                        rs_ps2.append(rs_ps)
                    for ct in range(NCT):
                        u_t = tup.tile([128, AC], f16, tag=f"u{ct}",
                                       name=f"u{ct}")
                        for c in range(AC // LQ):
                            cf = lo + c * LQ
                            nc.vector.tensor_tensor(
                                out=x_h[ct][:, cf:cf + LQ],
                                in0=x_h[ct][:, cf:cf + LQ],
                                in1=mu_ps2[c], op=OP.subtract)
                            nc.vector.tensor_tensor(
                                out=u_t[:, c * LQ:(c + 1) * LQ],
                                in0=x_h[ct][:, cf:cf + LQ],
                                in1=rs_ps2[c], op=OP.mult)
                        nc.scalar.activation(
                            out=xa_ap(ct, lo, AC), in_=u_t,
                            func=AF.Gelu, bias=lnb_c[ct], scale=lnw_c[ct])
                    for c in range(AC // LQ):
                        cf = lo + c * LQ
                        s_idx = cf // LQ
                        om_ps = psum_b.tile([128, LQ], f32, tag="bank",
                                            name="om_ps")
                        for ct in range(NCT):
                            nc.tensor.matmul(
                                om_ps[0:48, :], omwT[ct], xa_ap(ct, cf, LQ),
                                start=(ct == 0), stop=(ct == NCT - 1))
                        om_sb = sqsbp.tile([2 * G * K, LQ], f16, tag="om_sb",
                                           name="om_sb")
                        nc.scalar.activation(out=om_sb, in_=om_ps[0:48, :],
                                             func=AF.Identity,
                                             bias=bias48, scale=1.0)
                        # om -> qm_r repack for this lsub (one DMA)
                        nc.scalar.dma_start(
                            out=qm_r[s_idx * 8:(s_idx + 1) * 8, :].rearrange(
                                "g (qm k w) -> g qm k w", qm=2, k=K),
                            in_=om_sb)

        # ------------- banded coefficients (per half of the s rows) ------
        KL = K * LQ
        c_d = [cdp.tile([128, LQ], f16, tag=f"c{i}", name=f"c{i}")
               for i in range(ND)]

        def coeff_d(h, i):
            p0, p1 = (0, 128) if h is None else (h * 64, (h + 1) * 64)
            if True:
                a_t = coefp.tile([128, KL], f16, tag="a_t", name="a_t")
                nc.scalar.activation(out=a_t[p0:p1, :],
                                     in_=qm_r[p0:p1, 0:KL], func=AF.Abs,
                                     bias=negd_c[i][p0:p1, :], scale=1.0)
                nc.vector.tensor_scalar(out=a_t[p0:p1, :],
                                        in0=a_t[p0:p1, :], scalar1=-1.0,
                                        scalar2=1.0, op0=OP.mult, op1=OP.add)
                w_t = coefp.tile([128, KL], f16, tag="w_t", name="w_t")
                nc.vector.scalar_tensor_tensor(
                    out=w_t[p0:p1, :], in0=a_t[p0:p1, :], scalar=0.0,
                    in1=qm_r[p0:p1, KL:2 * KL], op0=OP.max, op1=OP.mult)
                nc.vector.tensor_add(out=c_d[i][p0:p1, :],
                                     in0=w_t[p0:p1, 0:LQ],
                                     in1=w_t[p0:p1, LQ:2 * LQ])
                nc.vector.tensor_add(out=c_d[i][p0:p1, :],
                                     in0=c_d[i][p0:p1, :],
                                     in1=w_t[p0:p1, 2 * LQ:3 * LQ])

        # ------------- phase 1: software-pipelined halves -------------
        st0 = psum_s.tile([128, 64], f32, tag="st", name="st0")
        for ci in range(8):
            emit_stats_chunk(st0, 0, ci)
        phase_statmath(0, st0)
        st1 = psum_s.tile([128, 64], f32, tag="st", name="st1")
        # apply h0 interleaved with stats h1
        rep_done = 0
        for b in range(2):
            emit_apply_block(0, b)
            for k in range(4):
                emit_stats_chunk(st1, 1, 4 * b + k)
            smax = ((b + 1) * BCH - WIN + HALO) // LQ + 1
            for s in range(rep_done, smax):
                emit_xar(s)
            rep_done = smax
        phase_statmath(1, st1)
        # apply h1
        for b in range(2):
            emit_apply_block(1, b)
            smax = ((LH + (b + 1) * BCH - WIN + HALO) // LQ + 1
                    if b == 0 else NS)
            for s in range(rep_done, smax):
                emit_xar(s)
            rep_done = smax
        for i in range(ND):
            coeff_d(None, i)

        # ------------- phase 4: banded aggregation -------------
        xa_r_v = xa_r[:].rearrange("p (cc w) -> p cc w", cc=GC)
        for cp in range(GC // CPG):
            accs = [psum_b.tile([128, LQ], f32, tag="bank", name=f"acc{j}")
                    for j in range(CPG)]
            for i in range(ND):
                d = D_LO + i
                tmp = tmpp.tile([128, CPG * LQ], f16, tag="tmp", name="tmp")
                cb = bass.AP(tensor=c_d[i].tensor, offset=c_d[i].offset,
                             ap=[c_d[i][:].ap[0], [0, CPG], [1, LQ]])
                nc.vector.tensor_tensor(
                    out=tmp[:].rearrange("p (c f) -> p c f", c=CPG),
                    in0=xa_r_v[:, cp * CPG:(cp + 1) * CPG,
                               HALO + d:HALO + d + LQ],
                    in1=cb, op=OP.mult)
                for j in range(CPG):
                    nc.tensor.matmul(accs[j], id128,
                                     tmp[:, j * LQ:(j + 1) * LQ],
                                     start=(i == 0), stop=(i == ND - 1))
            outc = outp.tile([128, CPG * LQ], f16, tag="outc", name="outc")
            for j in range(CPG):
                nc.scalar.copy(out=outc[:, j * LQ:(j + 1) * LQ],
                               in_=accs[j])
            nc.gpsimd.dma_start(
                out=out_d[:, cp * CPG * LQ:(cp + 1) * CPG * LQ], in_=outc)

    return nc


def _host_params(ln_w, ln_b, om_w, om_b):
    # om rows permuted to (g, qm, k); bias includes the conv grid (k-1)
    # folded into the offset rows.
    perm = np.zeros(2 * G * K, dtype=np.int64)
    bias = np.zeros(2 * G * K, dtype=np.float32)
    for g in range(G):
        for qm in range(2):
            for k in range(K):
                row = g * 6 + qm * 3 + k
                orig = qm * G * K + g * K + k
                perm[row] = orig
                bias[row] = om_b[orig] + (k - 1.0 if qm == 0 else 0.0)
    omP = om_w[perm]  # [48, C]
    sel2 = np.zeros((48, 2), np.float16)
    sel2[0, 0] = 1.0
    sel2[32, 1] = 1.0
    return {
        "lnw_col": np.asarray(ln_w, np.float32).reshape(C, 1),
        "lnb_col": np.asarray(ln_b, np.float32).reshape(C, 1),
        "onesc16": np.ones((128, 1), np.float16),
        "sel2": sel2,
        "om_wT": np.ascontiguousarray(omP.T).astype(np.float16),
        "bias48": bias.reshape(2 * G * K, 1),
        "id128h": np.eye(128, dtype=np.float16),
    }


def _unshuffle(buf):
    # buf [128, 16384] rows (s,g), cols (cp, j, f) -> [C, L]
    return np.ascontiguousarray(
        buf.reshape(NS, G, G, CPG, LQ).transpose(1, 2, 3, 0, 4)
        .reshape(C, L))


def kernel(x, ln_w, ln_b, om_w, om_b):
    _install_patch()
    from concourse.bass_utils import run_bass_kernel_spmd

    if "nc" not in _cache:
        _cache["nc"] = _build_module()
    nc = _cache["nc"]

    x = np.ascontiguousarray(np.asarray(x, dtype=np.float32))
    params = _host_params(np.asarray(ln_w, np.float32),
                          np.asarray(ln_b, np.float32),
                          np.asarray(om_w, np.float32),
                          np.asarray(om_b, np.float32))
    in_maps = [dict(params, x=x[n]) for n in range(N)]
    for _attempt in range(3):
        res = run_bass_kernel_spmd(nc, in_maps, core_ids=list(range(N)))
        out = np.stack([_unshuffle(res.results[n]["out"])
                        for n in range(N)], axis=0).astype(np.float32)
        if np.isfinite(out).all():
            return out
    return out


def run_traced(inputs):
    _install_patch()
    from concourse.bass_utils import run_bass_kernel_spmd
    if "nc" not in _cache:
        _cache["nc"] = _build_module()
    x = np.ascontiguousarray(np.asarray(inputs["x"], dtype=np.float32))
    params = _host_params(np.asarray(inputs["ln_w"], np.float32),
                          np.asarray(inputs["ln_b"], np.float32),
                          np.asarray(inputs["om_w"], np.float32),
                          np.asarray(inputs["om_b"], np.float32))
    in_maps = [dict(params, x=x[n]) for n in range(N)]
    return run_bass_kernel_spmd(_cache["nc"], in_maps,
                                core_ids=list(range(N)), trace=True)
